# revision 17
# baseline (speedup 1.0000x reference)
"""GatedGCN message-passing kernel for 8 Trainium2 NeuronCores (Bass/Tile).

Math (reference):
    newX = X @ Wn + bn
    agg  = segment_sum(a_vals[:,None] * newX[col], row, N)
    gate = sigmoid(X @ Wgi + bgi + agg @ Wgn + bgn)
    out  = agg * gate + X * (1 - gate)

Device strategy (per core, destination-sharded edges):
    Linearity lets the dense projection move past the aggregation:
        agg = (segsum(a * X_aug[col])) @ Wn_aug,    X_aug = [X | 1], Wn_aug = [Wn; bn]
    so the gather runs on raw bf16 X rows (SWDGE dma_gather, 256B rows) and the
    segment-sum is computed as one-hot matmuls accumulating in PSUM:
        per 128-edge chunk c of a 128-destination block b:
            S[e, d]    = (iota[d] == row_rel[e]) * a[e]          (DVE, 2 ops)
            aggX[:, b] += Xg_c.T @ S_c                            (PE, PSUM accum)
    giving aggX feature-major [97, dst]. All downstream dense math is then
        z    = Xo_aug @ Wgi_aug + aggX_aug @ (Wn_aug @ Wgn)      (biases folded)
        agg  = aggX_aug @ Wn_aug
        out  = agg * sigmoid(z) + Xo * sigmoid(-z)
    dma_gather uses int16 indices, so the node table is split in two 25000-row
    halves and each block's edges are grouped into per-half chunk sets with a
    globally uniform chunk count (C0/C1) so the SPMD program is identical on
    all 8 cores. Gather instructions are capped at 1024 indices (SWDGE
    descriptor-ring capacity) and rotate over 4 SWDGE queues, which runs the
    descriptor generation in parallel. Host work is index manipulation only.
"""

import numpy as np
import ml_dtypes

N = 50000
E = 800000
D = 96
DA = D + 1          # augmented features (ones column)
ROWE = 128          # padded row elements in the gather table (256B bf16)
NC_ = 8
NSH = N // NC_      # 6250 nodes per core
NHALF = N // 2      # table split for int16 gather indices
DW = 128            # destinations per block
NBLK = (NSH + DW - 1) // DW          # 49 blocks per core
NPAD = NBLK * DW                     # 6272 padded nodes per core
P = 128                              # edges per chunk
BPB = 7                              # blocks per gather batch
NBATCH = NBLK // BPB                 # 7 batches

_bf16 = ml_dtypes.bfloat16

_prog_cache = {}


def _host_prep(X, a_vals, Wn, bn, Wgi, bgi, Wgn, bgn, row, col):
    X = np.asarray(X, np.float32)
    a_vals = np.asarray(a_vals, np.float32)
    row = np.asarray(row, np.int64)
    col = np.asarray(col, np.int64)

    Wn_aug = np.vstack([np.asarray(Wn, np.float32), np.asarray(bn, np.float32)[None, :]])
    Wgi_aug = np.vstack([np.asarray(Wgi, np.float32),
                         (np.asarray(bgi, np.float32) + np.asarray(bgn, np.float32))[None, :]])
    W2_aug = Wn_aug @ np.asarray(Wgn, np.float32)

    X_pad = np.zeros((N, ROWE), np.float32)
    X_pad[:, :D] = X
    X_pad[:, D] = 1.0
    X_pad_bf = X_pad.astype(_bf16)
    xaug0 = np.ascontiguousarray(X_pad_bf[:NHALF])
    xaug1 = np.ascontiguousarray(X_pad_bf[NHALF:])

    core = row // NSH
    local = row - core * NSH
    blk = local // DW
    rr = local - blk * DW
    half = (col >= NHALF).astype(np.int64)

    # group edges by (core, block, half); get within-group positions
    gkey = (core * NBLK + blk) * 2 + half
    order = np.argsort(gkey, kind="stable")
    gk_sorted = gkey[order]
    counts = np.bincount(gk_sorted, minlength=NC_ * NBLK * 2)
    starts = np.concatenate([[0], np.cumsum(counts)])
    pos = np.arange(E, dtype=np.int64) - starts[gk_sorted]

    cnt2 = counts.reshape(NC_ * NBLK, 2)
    C0 = int(np.ceil(cnt2[:, 0].max() / P))
    C1 = int(np.ceil(cnt2[:, 1].max() / P))
    CPB = C0 + C1
    G = NBLK * CPB

    # global chunk index of chunk c of (block b, half h):
    #   batch bt = b // BPB, lb = b % BPB
    #   g = bt*BPB*CPB + (lb*C0 + c         if h == 0
    #                     BPB*C0 + lb*C1 + c if h == 1)
    b_all = blk[order]
    bt_all = b_all // BPB
    lb_all = b_all % BPB
    h_all = half[order]
    c_all = pos // P
    lane_all = pos - c_all * P
    g_all = bt_all * (BPB * CPB) + np.where(
        h_all == 0, lb_all * C0 + c_all, BPB * C0 + lb_all * C1 + c_all
    )
    slot_all = g_all * P + lane_all          # within-core slot in [0, G*P)

    col_l = (col - half * NHALF)[order].astype(np.int32)   # local table row
    rr_o = rr[order].astype(np.float32)
    av_o = a_vals[order].astype(np.float32)
    core_o = core[order]

    per_core = []
    for k in range(NC_):
        m = core_o == k
        slot = slot_all[m]
        idx_arr = np.zeros(G * P, np.int32)
        rr_arr = np.zeros(G * P, np.float32)
        av_arr = np.zeros(G * P, np.float32)
        idx_arr[slot] = col_l[m]
        rr_arr[slot] = rr_o[m]
        av_arr[slot] = av_o[m]

        rr2 = np.ascontiguousarray(rr_arr.reshape(G, P).T).astype(_bf16)
        av2 = np.ascontiguousarray(av_arr.reshape(G, P).T).astype(_bf16)

        # gather index stream: per batch, [h0 chunks (BPB*C0)] then [h1 chunks],
        # wrapped in 16 partitions and replicated across the 8 partition groups
        ix = idx_arr.reshape(G, P).astype(np.int16)        # [g, lane]
        wr = ix.reshape(G * P // 16, 16).T                 # [16, G*8]
        ixw = np.ascontiguousarray(np.tile(wr, (8, 1)))    # [128, G*8]

        xo = np.zeros((NPAD, DA), np.float32)
        xo[:NSH] = X_pad[k * NSH:(k + 1) * NSH, :DA]
        xo_fm = np.ascontiguousarray(xo.T).astype(_bf16)          # [97, 6272]
        xo_nm = np.ascontiguousarray(xo[:, :D])                   # [6272, 96] f32

        per_core.append({
            "xaug0": xaug0,
            "xaug1": xaug1,
            "ixw": ixw,
            "rr": rr2,
            "av": av2,
            "xofm": xo_fm,
            "xonm": xo_nm,
            "wgi": Wgi_aug.astype(_bf16),
            "w2": W2_aug.astype(_bf16),
            "wn": Wn_aug.astype(_bf16),
        })
    return per_core, (C0, C1)


def _build_program(C0, C1):
    import concourse.bass as bass
    import concourse.tile as tile
    from concourse import bacc, mybir

    CPB = C0 + C1
    G = NBLK * CPB
    NCHB = BPB * CPB                 # chunks per gather batch
    IXW = G * P // 16                # free dim of wrapped index tensor

    nc = bacc.Bacc("TRN2", target_bir_lowering=False, debug=False, num_devices=NC_,
                   num_swdge_queues=4)

    xaug0_d = nc.dram_tensor("xaug0", [NHALF, ROWE], mybir.dt.bfloat16, kind="ExternalInput")
    xaug1_d = nc.dram_tensor("xaug1", [NHALF, ROWE], mybir.dt.bfloat16, kind="ExternalInput")
    ixw_d = nc.dram_tensor("ixw", [P, IXW], mybir.dt.int16, kind="ExternalInput")
    rr_d = nc.dram_tensor("rr", [P, G], mybir.dt.bfloat16, kind="ExternalInput")
    av_d = nc.dram_tensor("av", [P, G], mybir.dt.bfloat16, kind="ExternalInput")
    xofm_d = nc.dram_tensor("xofm", [DA, NPAD], mybir.dt.bfloat16, kind="ExternalInput")
    xonm_d = nc.dram_tensor("xonm", [NPAD, D], mybir.dt.float32, kind="ExternalInput")
    wgi_d = nc.dram_tensor("wgi", [DA, D], mybir.dt.bfloat16, kind="ExternalInput")
    w2_d = nc.dram_tensor("w2", [DA, D], mybir.dt.bfloat16, kind="ExternalInput")
    wn_d = nc.dram_tensor("wn", [DA, D], mybir.dt.bfloat16, kind="ExternalInput")
    y_d = nc.dram_tensor("y", [NPAD, D], mybir.dt.float32, kind="ExternalOutput")

    with tile.TileContext(nc) as tc:
        with (
            tc.tile_pool(name="const", bufs=1) as cpool,
            tc.tile_pool(name="ix", bufs=2) as ixp,
            tc.tile_pool(name="xg", bufs=2) as xgp,
            tc.tile_pool(name="sa", bufs=1) as sap,
            tc.tile_pool(name="small", bufs=4) as smp,
            tc.tile_pool(name="segps", bufs=2, space="PSUM") as seg_psp,
            tc.tile_pool(name="zps", bufs=2, space="PSUM") as z_psp,
            tc.tile_pool(name="aggps", bufs=2, space="PSUM") as agg_psp,
        ):
            # ---- resident loads ----
            rr_t = cpool.tile([P, G], mybir.dt.bfloat16)
            nc.sync.dma_start(rr_t[:], rr_d.ap())
            av_t = cpool.tile([P, G], mybir.dt.bfloat16)
            nc.sync.dma_start(av_t[:], av_d.ap())
            xofm_t = cpool.tile([DA, NPAD], mybir.dt.bfloat16)
            nc.sync.dma_start(xofm_t[:], xofm_d.ap())
            xonm_t = cpool.tile([P, NBLK, D], mybir.dt.float32)
            nc.sync.dma_start(
                xonm_t[:], bass.AP(xonm_d, 0, [[D, P], [P * D, NBLK], [1, D]])
            )
            wgi_t = cpool.tile([DA, D], mybir.dt.bfloat16)
            nc.sync.dma_start(wgi_t[:], wgi_d.ap())
            w2_t = cpool.tile([DA, D], mybir.dt.bfloat16)
            nc.sync.dma_start(w2_t[:], w2_d.ap())
            wn_t = cpool.tile([DA, D], mybir.dt.bfloat16)
            nc.sync.dma_start(wn_t[:], wn_d.ap())

            iota_i = cpool.tile([P, DW], mybir.dt.int32)
            nc.gpsimd.iota(iota_i[:], pattern=[[1, DW]], base=0, channel_multiplier=0)
            iota_b = cpool.tile([P, DW], mybir.dt.bfloat16)
            nc.vector.tensor_copy(iota_b[:], iota_i[:])

            aggx_t = cpool.tile([DA, NPAD], mybir.dt.bfloat16)   # feature-major aggX
            outb_t = cpool.tile([P, NBLK, D], mybir.dt.float32)

            for batch in range(NBATCH):
                g0 = batch * NCHB
                ix_t = ixp.tile([P, NCHB * 8], mybir.dt.int16)
                nc.sync.dma_start(ix_t[:], ixw_d.ap()[:, g0 * 8:(g0 + NCHB) * 8])

                xg_t = xgp.tile([P, NCHB, ROWE], mybir.dt.bfloat16)
                n0 = BPB * C0 * P
                n1 = BPB * C1 * P
                # SWDGE descriptor ring holds 1024 entries; one gather
                # instruction must not exceed it. Rotating over the 4 SWDGE
                # queues parallelizes descriptor generation.
                MAXI = 512
                qi = 0
                for base, cnt, tab in ((0, n0, xaug0_d), (n0, n1, xaug1_d)):
                    off = 0
                    while off < cnt:
                        n = min(MAXI, cnt - off)
                        s = base + off
                        nc.gpsimd.dma_gather(
                            out_ap=xg_t[:, s // P:(s + n) // P, :], in_ap=tab.ap(),
                            idxs_ap=ix_t[:, s // 16:(s + n) // 16],
                            num_idxs=n, num_idxs_reg=n, elem_size=ROWE,
                            queue_num=qi % 4,
                        )
                        qi += 1
                        off += n

                sa_t = sap.tile([P, NCHB, DW], mybir.dt.bfloat16)
                iota_bc = bass.AP(iota_b[:].tensor, iota_b[:].offset,
                                  [iota_b[:].ap[0], [0, NCHB], [1, DW]])
                rr_sl = rr_t[:, g0:g0 + NCHB]
                rr_bc = bass.AP(rr_sl.tensor, rr_sl.offset,
                                [rr_sl.ap[0], [1, NCHB], [0, DW]])
                nc.vector.tensor_tensor(sa_t[:], iota_bc, rr_bc, mybir.AluOpType.is_equal)
                av_sl = av_t[:, g0:g0 + NCHB]
                av_bc = bass.AP(av_sl.tensor, av_sl.offset,
                                [av_sl.ap[0], [1, NCHB], [0, DW]])
                nc.vector.tensor_tensor(sa_t[:], sa_t[:], av_bc, mybir.AluOpType.mult)

                for lb in range(BPB):
                    j = batch * BPB + lb
                    # full-bank (2KB) psum tiles: accumulation-group zero
                    # regions are bank-granular, so tiles must not share banks
                    seg_ps = seg_psp.tile([P, 512], mybir.dt.float32, space="PSUM")
                    gls = [lb * C0 + c for c in range(C0)] + \
                          [BPB * C0 + lb * C1 + c for c in range(C1)]
                    for ci, gl in enumerate(gls):
                        nc.tensor.matmul(
                            out=seg_ps[:, :DW],
                            lhsT=xg_t[:, gl, :],
                            rhs=sa_t[:, gl, :],
                            start=(ci == 0),
                            stop=(ci == CPB - 1),
                        )
                    nc.scalar.copy(aggx_t[:, j * P:(j + 1) * P], seg_ps[:DA, :P])

                    # dense tail for this block of 128 nodes
                    z_ps = z_psp.tile([P, 512], mybir.dt.float32, space="PSUM")
                    agg_ps = agg_psp.tile([P, 512], mybir.dt.float32, space="PSUM")
                    sl = slice(j * DW, (j + 1) * DW)
                    nc.tensor.matmul(out=z_ps[:, :D], lhsT=xofm_t[:, sl],
                                     rhs=wgi_t[:], start=True, stop=False)
                    nc.tensor.matmul(out=z_ps[:, :D], lhsT=aggx_t[:, sl],
                                     rhs=w2_t[:], start=False, stop=True)
                    nc.tensor.matmul(out=agg_ps[:, :D], lhsT=aggx_t[:, sl],
                                     rhs=wn_t[:], start=True, stop=True)
                    g1_t = smp.tile([P, D], mybir.dt.float32)
                    nc.scalar.activation(g1_t[:], z_ps[:, :D], mybir.ActivationFunctionType.Sigmoid)
                    g2_t = smp.tile([P, D], mybir.dt.float32)
                    nc.scalar.activation(g2_t[:], z_ps[:, :D], mybir.ActivationFunctionType.Sigmoid,
                                         scale=-1.0)
                    nc.vector.tensor_tensor(g1_t[:], agg_ps[:, :D], g1_t[:], mybir.AluOpType.mult)
                    nc.vector.tensor_tensor(g2_t[:], xonm_t[:, j, :], g2_t[:], mybir.AluOpType.mult)
                    nc.vector.tensor_add(outb_t[:, j, :], g1_t[:], g2_t[:])

            nc.sync.dma_start(
                bass.AP(y_d, 0, [[D, P], [P * D, NBLK], [1, D]]), outb_t[:]
            )

    nc.compile()
    return nc


# test-harness hooks: set TRACE_TMPDIR to capture an NTFF profile on the next
# call; LAST_EXEC_NS then holds the profiled kernel execution time.
TRACE_TMPDIR = None
LAST_EXEC_NS = None


def kernel(X, a_vals, Wn, bn, Wgi, bgi, Wgn, bgn, row, col):
    global LAST_EXEC_NS
    from concourse.bass_utils import run_bass_kernel_spmd

    per_core, (C0, C1) = _host_prep(X, a_vals, Wn, bn, Wgi, bgi, Wgn, bgn, row, col)
    if (C0, C1) not in _prog_cache:
        _prog_cache[(C0, C1)] = _build_program(C0, C1)
    nc = _prog_cache[(C0, C1)]

    kwargs = {}
    if TRACE_TMPDIR is not None:
        kwargs = {"trace": True, "tmpdir": TRACE_TMPDIR}
    res = run_bass_kernel_spmd(nc, per_core, core_ids=list(range(NC_)), **kwargs)
    LAST_EXEC_NS = res.exec_time_ns
    out = np.empty((N, D), np.float32)
    for k in range(NC_):
        out[k * NSH:(k + 1) * NSH] = res.results[k]["y"][:NSH]
    return out


# revision 18
# speedup vs baseline: 1.1266x; 1.1266x over previous
"""GatedGCN message-passing kernel for 8 Trainium2 NeuronCores (Bass/Tile).

Math (reference):
    newX = X @ Wn + bn
    agg  = segment_sum(a_vals[:,None] * newX[col], row, N)
    gate = sigmoid(X @ Wgi + bgi + agg @ Wgn + bgn)
    out  = agg * gate + X * (1 - gate)

Device strategy (per core, destination-sharded edges):
    Linearity lets the dense projection move past the aggregation:
        agg = (segsum(a * X_aug[col])) @ Wn_aug,    X_aug = [X | 1], Wn_aug = [Wn; bn]
    so the gather runs on raw bf16 X rows (SWDGE dma_gather, 256B rows) and the
    segment-sum is computed as one-hot matmuls accumulating in PSUM:
        per 128-edge chunk c of a 128-destination block b:
            S[e, d]    = (iota[d] == row_rel[e]) * a[e]          (DVE, 2 ops)
            aggX[:, b] += Xg_c.T @ S_c                            (PE, PSUM accum)
    giving aggX feature-major [97, dst]. All downstream dense math is then
        z    = Xo_aug @ Wgi_aug + aggX_aug @ (Wn_aug @ Wgn)      (biases folded)
        agg  = aggX_aug @ Wn_aug
        out  = agg * sigmoid(z) + Xo * sigmoid(-z)
    dma_gather uses int16 indices, so the node table is split in two 25000-row
    halves and each block's edges are grouped into per-half chunk sets with a
    globally uniform chunk count (C0/C1) so the SPMD program is identical on
    all 8 cores. Gather instructions are capped at 1024 indices (SWDGE
    descriptor-ring capacity) and rotate over 4 SWDGE queues, which runs the
    descriptor generation in parallel. Host work is index manipulation only.
"""

import numpy as np
import ml_dtypes

N = 50000
E = 800000
D = 96
DA = D + 1          # augmented features (ones column)
ROWE = 128          # padded row elements in the gather table (256B bf16)
NC_ = 8
NSH = N // NC_      # 6250 nodes per core
NHALF = N // 2      # table split for int16 gather indices
DW = 128            # destinations per block
NBLK = (NSH + DW - 1) // DW          # 49 blocks per core
NPAD = NBLK * DW                     # 6272 padded nodes per core
P = 128                              # edges per chunk
BPB = 7                              # blocks per gather batch
NBATCH = NBLK // BPB                 # 7 batches

_bf16 = ml_dtypes.bfloat16

_prog_cache = {}


def _host_prep(X, a_vals, Wn, bn, Wgi, bgi, Wgn, bgn, row, col):
    X = np.asarray(X, np.float32)
    a_vals = np.asarray(a_vals, np.float32)
    row = np.asarray(row, np.int64)
    col = np.asarray(col, np.int64)

    Wn_aug = np.vstack([np.asarray(Wn, np.float32), np.asarray(bn, np.float32)[None, :]])
    Wgi_aug = np.vstack([np.asarray(Wgi, np.float32),
                         (np.asarray(bgi, np.float32) + np.asarray(bgn, np.float32))[None, :]])
    W2_aug = Wn_aug @ np.asarray(Wgn, np.float32)

    X_pad = np.zeros((N, ROWE), np.float32)
    X_pad[:, :D] = X
    X_pad[:, D] = 1.0
    X_pad_bf = X_pad.astype(_bf16)
    xaug0 = np.ascontiguousarray(X_pad_bf[:NHALF])
    xaug1 = np.ascontiguousarray(X_pad_bf[NHALF:])

    core = row // NSH
    local = row - core * NSH
    blk = local // DW
    rr = local - blk * DW
    half = (col >= NHALF).astype(np.int64)

    # group edges by (core, block, half); get within-group positions
    gkey = (core * NBLK + blk) * 2 + half
    order = np.argsort(gkey, kind="stable")
    gk_sorted = gkey[order]
    counts = np.bincount(gk_sorted, minlength=NC_ * NBLK * 2)
    starts = np.concatenate([[0], np.cumsum(counts)])
    pos = np.arange(E, dtype=np.int64) - starts[gk_sorted]

    cnt2 = counts.reshape(NC_ * NBLK, 2)
    C0 = int(np.ceil(cnt2[:, 0].max() / P))
    C1 = int(np.ceil(cnt2[:, 1].max() / P))
    CPB = C0 + C1
    G = NBLK * CPB

    # global chunk index of chunk c of (block b, half h):
    #   batch bt = b // BPB, lb = b % BPB
    #   g = bt*BPB*CPB + (lb*C0 + c         if h == 0
    #                     BPB*C0 + lb*C1 + c if h == 1)
    b_all = blk[order]
    bt_all = b_all // BPB
    lb_all = b_all % BPB
    h_all = half[order]
    c_all = pos // P
    lane_all = pos - c_all * P
    g_all = bt_all * (BPB * CPB) + np.where(
        h_all == 0, lb_all * C0 + c_all, BPB * C0 + lb_all * C1 + c_all
    )
    slot_all = g_all * P + lane_all          # within-core slot in [0, G*P)

    col_l = (col - half * NHALF)[order].astype(np.int32)   # local table row
    rr_o = rr[order].astype(np.float32)
    av_o = a_vals[order].astype(np.float32)
    core_o = core[order]

    per_core = []
    for k in range(NC_):
        m = core_o == k
        slot = slot_all[m]
        idx_arr = np.zeros(G * P, np.int32)
        rr_arr = np.zeros(G * P, np.float32)
        av_arr = np.zeros(G * P, np.float32)
        idx_arr[slot] = col_l[m]
        rr_arr[slot] = rr_o[m]
        av_arr[slot] = av_o[m]

        rr2 = np.ascontiguousarray(rr_arr.reshape(G, P).T).astype(_bf16)
        av2 = np.ascontiguousarray(av_arr.reshape(G, P).T).astype(_bf16)

        # gather index stream: per batch, [h0 chunks (BPB*C0)] then [h1 chunks],
        # wrapped in 16 partitions and replicated across the 8 partition groups
        ix = idx_arr.reshape(G, P).astype(np.int16)        # [g, lane]
        wr = ix.reshape(G * P // 16, 16).T                 # [16, G*8]
        ixw = np.ascontiguousarray(np.tile(wr, (8, 1)))    # [128, G*8]

        xo = np.zeros((NPAD, DA), np.float32)
        xo[:NSH] = X_pad[k * NSH:(k + 1) * NSH, :DA]
        xo_fm = np.ascontiguousarray(xo.T).astype(_bf16)          # [97, 6272]
        xo_nm = np.ascontiguousarray(xo[:, :D])                   # [6272, 96] f32

        per_core.append({
            "xaug0": xaug0,
            "xaug1": xaug1,
            "ixw": ixw,
            "rr": rr2,
            "av": av2,
            "xofm": xo_fm,
            "xonm": xo_nm,
            "wgi": Wgi_aug.astype(_bf16),
            "w2": W2_aug.astype(_bf16),
            "wn": Wn_aug.astype(_bf16),
        })
    return per_core, (C0, C1)


def _build_program(C0, C1):
    import concourse.bass as bass
    import concourse.tile as tile
    from concourse import bacc, mybir

    CPB = C0 + C1
    G = NBLK * CPB
    NCHB = BPB * CPB                 # chunks per gather batch
    IXW = G * P // 16                # free dim of wrapped index tensor

    nc = bacc.Bacc("TRN2", target_bir_lowering=False, debug=False, num_devices=NC_,
                   num_swdge_queues=4)

    xaug0_d = nc.dram_tensor("xaug0", [NHALF, ROWE], mybir.dt.bfloat16, kind="ExternalInput")
    xaug1_d = nc.dram_tensor("xaug1", [NHALF, ROWE], mybir.dt.bfloat16, kind="ExternalInput")
    ixw_d = nc.dram_tensor("ixw", [P, IXW], mybir.dt.int16, kind="ExternalInput")
    rr_d = nc.dram_tensor("rr", [P, G], mybir.dt.bfloat16, kind="ExternalInput")
    av_d = nc.dram_tensor("av", [P, G], mybir.dt.bfloat16, kind="ExternalInput")
    xofm_d = nc.dram_tensor("xofm", [DA, NPAD], mybir.dt.bfloat16, kind="ExternalInput")
    xonm_d = nc.dram_tensor("xonm", [NPAD, D], mybir.dt.float32, kind="ExternalInput")
    wgi_d = nc.dram_tensor("wgi", [DA, D], mybir.dt.bfloat16, kind="ExternalInput")
    w2_d = nc.dram_tensor("w2", [DA, D], mybir.dt.bfloat16, kind="ExternalInput")
    wn_d = nc.dram_tensor("wn", [DA, D], mybir.dt.bfloat16, kind="ExternalInput")
    y_d = nc.dram_tensor("y", [NPAD, D], mybir.dt.float32, kind="ExternalOutput")

    with tile.TileContext(nc) as tc:
        with (
            tc.tile_pool(name="const", bufs=1) as cpool,
            tc.tile_pool(name="ix", bufs=2) as ixp,
            tc.tile_pool(name="xg", bufs=2) as xgp,
            tc.tile_pool(name="sa", bufs=1) as sap,
            tc.tile_pool(name="small", bufs=4) as smp,
            tc.tile_pool(name="segps", bufs=2, space="PSUM") as seg_psp,
            tc.tile_pool(name="zps", bufs=2, space="PSUM") as z_psp,
            tc.tile_pool(name="aggps", bufs=2, space="PSUM") as agg_psp,
        ):
            # ---- resident loads ----
            rr_t = cpool.tile([P, G], mybir.dt.bfloat16)
            nc.sync.dma_start(rr_t[:], rr_d.ap())
            av_t = cpool.tile([P, G], mybir.dt.bfloat16)
            nc.sync.dma_start(av_t[:], av_d.ap())
            xofm_t = cpool.tile([DA, NPAD], mybir.dt.bfloat16)
            nc.sync.dma_start(xofm_t[:], xofm_d.ap())
            xonm_t = cpool.tile([P, NBLK, D], mybir.dt.float32)
            nc.sync.dma_start(
                xonm_t[:], bass.AP(xonm_d, 0, [[D, P], [P * D, NBLK], [1, D]])
            )
            wgi_t = cpool.tile([DA, D], mybir.dt.bfloat16)
            nc.sync.dma_start(wgi_t[:], wgi_d.ap())
            w2_t = cpool.tile([DA, D], mybir.dt.bfloat16)
            nc.sync.dma_start(w2_t[:], w2_d.ap())
            wn_t = cpool.tile([DA, D], mybir.dt.bfloat16)
            nc.sync.dma_start(wn_t[:], wn_d.ap())

            iota_i = cpool.tile([P, DW], mybir.dt.int32)
            nc.gpsimd.iota(iota_i[:], pattern=[[1, DW]], base=0, channel_multiplier=0)
            iota_b = cpool.tile([P, DW], mybir.dt.bfloat16)
            nc.vector.tensor_copy(iota_b[:], iota_i[:])

            aggx_t = cpool.tile([DA, NPAD], mybir.dt.bfloat16)   # feature-major aggX
            outb_t = cpool.tile([P, NBLK, D], mybir.dt.float32)

            for batch in range(NBATCH):
                g0 = batch * NCHB
                ix_t = ixp.tile([P, NCHB * 8], mybir.dt.int16)
                nc.sync.dma_start(ix_t[:], ixw_d.ap()[:, g0 * 8:(g0 + NCHB) * 8])

                xg_t = xgp.tile([P, NCHB, ROWE], mybir.dt.bfloat16)
                n0 = BPB * C0 * P
                n1 = BPB * C1 * P
                # SWDGE descriptor ring holds 1024 entries; one gather
                # instruction must not exceed it. Rotating over the 4 SWDGE
                # queues parallelizes descriptor generation.
                MAXI = 1024
                qi = 0
                for base, cnt, tab in ((0, n0, xaug0_d), (n0, n1, xaug1_d)):
                    off = 0
                    while off < cnt:
                        n = min(MAXI, cnt - off)
                        s = base + off
                        nc.gpsimd.dma_gather(
                            out_ap=xg_t[:, s // P:(s + n) // P, :], in_ap=tab.ap(),
                            idxs_ap=ix_t[:, s // 16:(s + n) // 16],
                            num_idxs=n, num_idxs_reg=n, elem_size=ROWE,
                            queue_num=qi % 4,
                        )
                        qi += 1
                        off += n

                sa_t = sap.tile([P, NCHB, DW], mybir.dt.bfloat16)
                iota_bc = bass.AP(iota_b[:].tensor, iota_b[:].offset,
                                  [iota_b[:].ap[0], [0, NCHB], [1, DW]])
                rr_sl = rr_t[:, g0:g0 + NCHB]
                rr_bc = bass.AP(rr_sl.tensor, rr_sl.offset,
                                [rr_sl.ap[0], [1, NCHB], [0, DW]])
                nc.vector.tensor_tensor(sa_t[:], iota_bc, rr_bc, mybir.AluOpType.is_equal)
                av_sl = av_t[:, g0:g0 + NCHB]
                av_bc = bass.AP(av_sl.tensor, av_sl.offset,
                                [av_sl.ap[0], [1, NCHB], [0, DW]])
                nc.vector.tensor_tensor(sa_t[:], sa_t[:], av_bc, mybir.AluOpType.mult)

                for lb in range(BPB):
                    j = batch * BPB + lb
                    # full-bank (2KB) psum tiles: accumulation-group zero
                    # regions are bank-granular, so tiles must not share banks
                    seg_ps = seg_psp.tile([P, 512], mybir.dt.float32, space="PSUM")
                    gls = [lb * C0 + c for c in range(C0)] + \
                          [BPB * C0 + lb * C1 + c for c in range(C1)]
                    for ci, gl in enumerate(gls):
                        nc.tensor.matmul(
                            out=seg_ps[:, :DW],
                            lhsT=xg_t[:, gl, :],
                            rhs=sa_t[:, gl, :],
                            start=(ci == 0),
                            stop=(ci == CPB - 1),
                        )
                    nc.scalar.copy(aggx_t[:, j * P:(j + 1) * P], seg_ps[:DA, :P])

                    # dense tail for this block of 128 nodes
                    z_ps = z_psp.tile([P, 512], mybir.dt.float32, space="PSUM")
                    agg_ps = agg_psp.tile([P, 512], mybir.dt.float32, space="PSUM")
                    sl = slice(j * DW, (j + 1) * DW)
                    nc.tensor.matmul(out=z_ps[:, :D], lhsT=xofm_t[:, sl],
                                     rhs=wgi_t[:], start=True, stop=False)
                    nc.tensor.matmul(out=z_ps[:, :D], lhsT=aggx_t[:, sl],
                                     rhs=w2_t[:], start=False, stop=True)
                    nc.tensor.matmul(out=agg_ps[:, :D], lhsT=aggx_t[:, sl],
                                     rhs=wn_t[:], start=True, stop=True)
                    g1_t = smp.tile([P, D], mybir.dt.float32)
                    nc.scalar.activation(g1_t[:], z_ps[:, :D], mybir.ActivationFunctionType.Sigmoid)
                    g2_t = smp.tile([P, D], mybir.dt.float32)
                    nc.scalar.activation(g2_t[:], z_ps[:, :D], mybir.ActivationFunctionType.Sigmoid,
                                         scale=-1.0)
                    nc.vector.tensor_tensor(g1_t[:], agg_ps[:, :D], g1_t[:], mybir.AluOpType.mult)
                    nc.vector.tensor_tensor(g2_t[:], xonm_t[:, j, :], g2_t[:], mybir.AluOpType.mult)
                    nc.vector.tensor_add(outb_t[:, j, :], g1_t[:], g2_t[:])

            nc.sync.dma_start(
                bass.AP(y_d, 0, [[D, P], [P * D, NBLK], [1, D]]), outb_t[:]
            )

    nc.compile()
    return nc


# test-harness hooks: set TRACE_TMPDIR to capture an NTFF profile on the next
# call; LAST_EXEC_NS then holds the profiled kernel execution time.
TRACE_TMPDIR = None
LAST_EXEC_NS = None


def kernel(X, a_vals, Wn, bn, Wgi, bgi, Wgn, bgn, row, col):
    global LAST_EXEC_NS
    from concourse.bass_utils import run_bass_kernel_spmd

    per_core, (C0, C1) = _host_prep(X, a_vals, Wn, bn, Wgi, bgi, Wgn, bgn, row, col)
    if (C0, C1) not in _prog_cache:
        _prog_cache[(C0, C1)] = _build_program(C0, C1)
    nc = _prog_cache[(C0, C1)]

    kwargs = {}
    if TRACE_TMPDIR is not None:
        kwargs = {"trace": True, "tmpdir": TRACE_TMPDIR}
    res = run_bass_kernel_spmd(nc, per_core, core_ids=list(range(NC_)), **kwargs)
    LAST_EXEC_NS = res.exec_time_ns
    out = np.empty((N, D), np.float32)
    for k in range(NC_):
        out[k * NSH:(k + 1) * NSH] = res.results[k]["y"][:NSH]
    return out


# revision 19
# speedup vs baseline: 1.2253x; 1.0876x over previous
"""GatedGCN message-passing kernel for 8 Trainium2 NeuronCores (Bass/Tile).

Math (reference):
    newX = X @ Wn + bn
    agg  = segment_sum(a_vals[:,None] * newX[col], row, N)
    gate = sigmoid(X @ Wgi + bgi + agg @ Wgn + bgn)
    out  = agg * gate + X * (1 - gate)

Device strategy (per core, destination-sharded edges):
    Linearity lets the dense projection move past the aggregation:
        agg = (segsum(a * X_aug[col])) @ Wn_aug,    X_aug = [X | 1], Wn_aug = [Wn; bn]
    so the gather runs on raw bf16 X rows (SWDGE dma_gather, 256B rows) and the
    segment-sum is computed as one-hot matmuls accumulating in PSUM:
        per 128-edge chunk c of a 128-destination block b:
            S[e, d]    = (iota[d] == row_rel[e]) * a[e]          (DVE, 2 ops)
            aggX[:, b] += Xg_c.T @ S_c                            (PE, PSUM accum)
    giving aggX feature-major [97, dst]. All downstream dense math is then
        z    = Xo_aug @ Wgi_aug + aggX_aug @ (Wn_aug @ Wgn)      (biases folded)
        agg  = aggX_aug @ Wn_aug
        out  = agg * sigmoid(z) + Xo * sigmoid(-z)
    dma_gather uses int16 indices, so the node table is split in two 25000-row
    halves and each block's edges are grouped into per-half chunk sets with a
    globally uniform chunk count (C0/C1) so the SPMD program is identical on
    all 8 cores. Gather instructions are capped at 1024 indices (SWDGE
    descriptor-ring capacity) and rotate over 4 SWDGE queues, which runs the
    descriptor generation in parallel. Host work is index manipulation only.
"""

import numpy as np
import ml_dtypes

N = 50000
E = 800000
D = 96
DA = D + 1          # augmented features (ones column)
ROWE = 128          # padded row elements in the gather table (256B bf16)
NC_ = 8
NSH = N // NC_      # 6250 nodes per core
NHALF = N // 2      # table split for int16 gather indices
DW = 128            # destinations per block
NBLK = (NSH + DW - 1) // DW          # 49 blocks per core
NPAD = NBLK * DW                     # 6272 padded nodes per core
P = 128                              # edges per chunk
BPB = 7                              # blocks per gather batch
NBATCH = NBLK // BPB                 # 7 batches

_bf16 = ml_dtypes.bfloat16

_prog_cache = {}


def _host_prep(X, a_vals, Wn, bn, Wgi, bgi, Wgn, bgn, row, col):
    X = np.asarray(X, np.float32)
    a_vals = np.asarray(a_vals, np.float32)
    row = np.asarray(row, np.int64)
    col = np.asarray(col, np.int64)

    Wn_aug = np.vstack([np.asarray(Wn, np.float32), np.asarray(bn, np.float32)[None, :]])
    Wgi_aug = np.vstack([np.asarray(Wgi, np.float32),
                         (np.asarray(bgi, np.float32) + np.asarray(bgn, np.float32))[None, :]])
    W2_aug = Wn_aug @ np.asarray(Wgn, np.float32)

    X_pad = np.zeros((N, ROWE), np.float32)
    X_pad[:, :D] = X
    X_pad[:, D] = 1.0
    X_pad_bf = X_pad.astype(_bf16)
    xaug0 = np.ascontiguousarray(X_pad_bf[:NHALF])
    xaug1 = np.ascontiguousarray(X_pad_bf[NHALF:])

    core = row // NSH
    local = row - core * NSH
    blk = local // DW
    rr = local - blk * DW
    half = (col >= NHALF).astype(np.int64)

    # group edges by (core, block, half); get within-group positions
    gkey = (core * NBLK + blk) * 2 + half
    order = np.argsort(gkey, kind="stable")
    gk_sorted = gkey[order]
    counts = np.bincount(gk_sorted, minlength=NC_ * NBLK * 2)
    starts = np.concatenate([[0], np.cumsum(counts)])
    pos = np.arange(E, dtype=np.int64) - starts[gk_sorted]

    cnt2 = counts.reshape(NC_ * NBLK, 2)
    C0 = int(np.ceil(cnt2[:, 0].max() / P))
    C1 = int(np.ceil(cnt2[:, 1].max() / P))
    CPB = C0 + C1
    G = NBLK * CPB

    # global chunk index of chunk c of (block b, half h):
    #   batch bt = b // BPB, lb = b % BPB
    #   g = bt*BPB*CPB + (lb*C0 + c         if h == 0
    #                     BPB*C0 + lb*C1 + c if h == 1)
    b_all = blk[order]
    bt_all = b_all // BPB
    lb_all = b_all % BPB
    h_all = half[order]
    c_all = pos // P
    lane_all = pos - c_all * P
    g_all = bt_all * (BPB * CPB) + np.where(
        h_all == 0, lb_all * C0 + c_all, BPB * C0 + lb_all * C1 + c_all
    )
    slot_all = g_all * P + lane_all          # within-core slot in [0, G*P)

    col_l = (col - half * NHALF)[order].astype(np.int32)   # local table row
    rr_o = rr[order].astype(np.float32)
    av_o = a_vals[order].astype(np.float32)
    core_o = core[order]

    per_core = []
    for k in range(NC_):
        m = core_o == k
        slot = slot_all[m]
        idx_arr = np.zeros(G * P, np.int32)
        rr_arr = np.zeros(G * P, np.float32)
        av_arr = np.zeros(G * P, np.float32)
        idx_arr[slot] = col_l[m]
        rr_arr[slot] = rr_o[m]
        av_arr[slot] = av_o[m]

        rr2 = np.ascontiguousarray(rr_arr.reshape(G, P).T).astype(_bf16)
        av2 = np.ascontiguousarray(av_arr.reshape(G, P).T).astype(_bf16)

        # gather index stream: per batch, [h0 chunks (BPB*C0)] then [h1 chunks],
        # wrapped in 16 partitions and replicated across the 8 partition groups
        ix = idx_arr.reshape(G, P).astype(np.int16)        # [g, lane]
        wr = ix.reshape(G * P // 16, 16).T                 # [16, G*8]
        ixw = np.ascontiguousarray(np.tile(wr, (8, 1)))    # [128, G*8]

        xo = np.zeros((NPAD, DA), np.float32)
        xo[:NSH] = X_pad[k * NSH:(k + 1) * NSH, :DA]
        xo_fm = np.ascontiguousarray(xo.T).astype(_bf16)          # [97, 6272]
        xo_nm = np.ascontiguousarray(xo[:, :D])                   # [6272, 96] f32

        per_core.append({
            "xaug0": xaug0,
            "xaug1": xaug1,
            "ixw": ixw,
            "rr": rr2,
            "av": av2,
            "xofm": xo_fm,
            "xonm": xo_nm,
            "wgi": Wgi_aug.astype(_bf16),
            "w2": W2_aug.astype(_bf16),
            "wn": Wn_aug.astype(_bf16),
        })
    return per_core, (C0, C1)


def _build_program(C0, C1):
    import concourse.bass as bass
    import concourse.tile as tile
    from concourse import bacc, mybir

    CPB = C0 + C1
    G = NBLK * CPB
    NCHB = BPB * CPB                 # chunks per gather batch
    IXW = G * P // 16                # free dim of wrapped index tensor

    nc = bacc.Bacc("TRN2", target_bir_lowering=False, debug=False, num_devices=NC_,
                   num_swdge_queues=4)

    xaug0_d = nc.dram_tensor("xaug0", [NHALF, ROWE], mybir.dt.bfloat16, kind="ExternalInput")
    xaug1_d = nc.dram_tensor("xaug1", [NHALF, ROWE], mybir.dt.bfloat16, kind="ExternalInput")
    ixw_d = nc.dram_tensor("ixw", [P, IXW], mybir.dt.int16, kind="ExternalInput")
    rr_d = nc.dram_tensor("rr", [P, G], mybir.dt.bfloat16, kind="ExternalInput")
    av_d = nc.dram_tensor("av", [P, G], mybir.dt.bfloat16, kind="ExternalInput")
    xofm_d = nc.dram_tensor("xofm", [DA, NPAD], mybir.dt.bfloat16, kind="ExternalInput")
    xonm_d = nc.dram_tensor("xonm", [NPAD, D], mybir.dt.float32, kind="ExternalInput")
    wgi_d = nc.dram_tensor("wgi", [DA, D], mybir.dt.bfloat16, kind="ExternalInput")
    w2_d = nc.dram_tensor("w2", [DA, D], mybir.dt.bfloat16, kind="ExternalInput")
    wn_d = nc.dram_tensor("wn", [DA, D], mybir.dt.bfloat16, kind="ExternalInput")
    y_d = nc.dram_tensor("y", [NPAD, D], mybir.dt.float32, kind="ExternalOutput")

    with tile.TileContext(nc) as tc:
        with (
            tc.tile_pool(name="const", bufs=1) as cpool,
            tc.tile_pool(name="ix", bufs=2) as ixp,
            tc.tile_pool(name="xg", bufs=2) as xgp,
            tc.tile_pool(name="sa", bufs=1) as sap,
            tc.tile_pool(name="small", bufs=4) as smp,
            tc.tile_pool(name="segps", bufs=2, space="PSUM") as seg_psp,
            tc.tile_pool(name="zps", bufs=2, space="PSUM") as z_psp,
            tc.tile_pool(name="aggps", bufs=2, space="PSUM") as agg_psp,
        ):
            # ---- resident loads ----
            rr_t = cpool.tile([P, G], mybir.dt.bfloat16)
            nc.scalar.dma_start(rr_t[:], rr_d.ap())
            av_t = cpool.tile([P, G], mybir.dt.bfloat16)
            nc.scalar.dma_start(av_t[:], av_d.ap())
            xofm_t = cpool.tile([DA, NPAD], mybir.dt.bfloat16)
            nc.scalar.dma_start(xofm_t[:], xofm_d.ap())
            xonm_t = cpool.tile([P, NBLK, D], mybir.dt.float32)
            nc.scalar.dma_start(
                xonm_t[:], bass.AP(xonm_d, 0, [[D, P], [P * D, NBLK], [1, D]])
            )
            wgi_t = cpool.tile([DA, D], mybir.dt.bfloat16)
            nc.scalar.dma_start(wgi_t[:], wgi_d.ap())
            w2_t = cpool.tile([DA, D], mybir.dt.bfloat16)
            nc.scalar.dma_start(w2_t[:], w2_d.ap())
            wn_t = cpool.tile([DA, D], mybir.dt.bfloat16)
            nc.scalar.dma_start(wn_t[:], wn_d.ap())

            iota_i = cpool.tile([P, DW], mybir.dt.int32)
            nc.gpsimd.iota(iota_i[:], pattern=[[1, DW]], base=0, channel_multiplier=0)
            iota_b = cpool.tile([P, DW], mybir.dt.bfloat16)
            nc.vector.tensor_copy(iota_b[:], iota_i[:])

            aggx_t = cpool.tile([DA, NPAD], mybir.dt.bfloat16)   # feature-major aggX
            outb_t = cpool.tile([P, NBLK, D], mybir.dt.float32)

            qi = 0
            for batch in range(NBATCH):
                g0 = batch * NCHB
                ix_t = ixp.tile([P, NCHB * 8], mybir.dt.int16)
                nc.sync.dma_start(ix_t[:], ixw_d.ap()[:, g0 * 8:(g0 + NCHB) * 8])

                xg_t = xgp.tile([P, NCHB, ROWE], mybir.dt.bfloat16)
                n0 = BPB * C0 * P
                n1 = BPB * C1 * P
                # SWDGE descriptor ring holds 1024 entries; one gather
                # instruction must not exceed it. Rotating over the 4 SWDGE
                # queues parallelizes descriptor generation.
                MAXI = 1024
                for base, cnt, tab in ((0, n0, xaug0_d), (n0, n1, xaug1_d)):
                    off = 0
                    while off < cnt:
                        n = min(MAXI, cnt - off)
                        s = base + off
                        nc.gpsimd.dma_gather(
                            out_ap=xg_t[:, s // P:(s + n) // P, :], in_ap=tab.ap(),
                            idxs_ap=ix_t[:, s // 16:(s + n) // 16],
                            num_idxs=n, num_idxs_reg=n, elem_size=ROWE,
                            queue_num=qi % 4,
                        )
                        qi += 1
                        off += n

                sa_t = sap.tile([P, NCHB, DW], mybir.dt.bfloat16)
                iota_bc = bass.AP(iota_b[:].tensor, iota_b[:].offset,
                                  [iota_b[:].ap[0], [0, NCHB], [1, DW]])
                rr_sl = rr_t[:, g0:g0 + NCHB]
                rr_bc = bass.AP(rr_sl.tensor, rr_sl.offset,
                                [rr_sl.ap[0], [1, NCHB], [0, DW]])
                nc.vector.tensor_tensor(sa_t[:], iota_bc, rr_bc, mybir.AluOpType.is_equal)
                av_sl = av_t[:, g0:g0 + NCHB]
                av_bc = bass.AP(av_sl.tensor, av_sl.offset,
                                [av_sl.ap[0], [1, NCHB], [0, DW]])
                nc.vector.tensor_tensor(sa_t[:], sa_t[:], av_bc, mybir.AluOpType.mult)

                for lb in range(BPB):
                    j = batch * BPB + lb
                    # full-bank (2KB) psum tiles: accumulation-group zero
                    # regions are bank-granular, so tiles must not share banks
                    seg_ps = seg_psp.tile([P, 512], mybir.dt.float32, space="PSUM")
                    gls = [lb * C0 + c for c in range(C0)] + \
                          [BPB * C0 + lb * C1 + c for c in range(C1)]
                    for ci, gl in enumerate(gls):
                        nc.tensor.matmul(
                            out=seg_ps[:, :DW],
                            lhsT=xg_t[:, gl, :],
                            rhs=sa_t[:, gl, :],
                            start=(ci == 0),
                            stop=(ci == CPB - 1),
                        )
                    nc.scalar.copy(aggx_t[:, j * P:(j + 1) * P], seg_ps[:DA, :P])

                    # dense tail for this block of 128 nodes
                    z_ps = z_psp.tile([P, 512], mybir.dt.float32, space="PSUM")
                    agg_ps = agg_psp.tile([P, 512], mybir.dt.float32, space="PSUM")
                    sl = slice(j * DW, (j + 1) * DW)
                    nc.tensor.matmul(out=z_ps[:, :D], lhsT=xofm_t[:, sl],
                                     rhs=wgi_t[:], start=True, stop=False)
                    nc.tensor.matmul(out=z_ps[:, :D], lhsT=aggx_t[:, sl],
                                     rhs=w2_t[:], start=False, stop=True)
                    nc.tensor.matmul(out=agg_ps[:, :D], lhsT=aggx_t[:, sl],
                                     rhs=wn_t[:], start=True, stop=True)
                    g1_t = smp.tile([P, D], mybir.dt.float32)
                    nc.scalar.activation(g1_t[:], z_ps[:, :D], mybir.ActivationFunctionType.Sigmoid)
                    g2_t = smp.tile([P, D], mybir.dt.float32)
                    nc.scalar.activation(g2_t[:], z_ps[:, :D], mybir.ActivationFunctionType.Sigmoid,
                                         scale=-1.0)
                    nc.vector.tensor_tensor(g1_t[:], agg_ps[:, :D], g1_t[:], mybir.AluOpType.mult)
                    nc.vector.tensor_tensor(g2_t[:], xonm_t[:, j, :], g2_t[:], mybir.AluOpType.mult)
                    nc.vector.tensor_add(outb_t[:, j, :], g1_t[:], g2_t[:])

            nc.sync.dma_start(
                bass.AP(y_d, 0, [[D, P], [P * D, NBLK], [1, D]]), outb_t[:]
            )

    nc.compile()
    return nc


# test-harness hooks: set TRACE_TMPDIR to capture an NTFF profile on the next
# call; LAST_EXEC_NS then holds the profiled kernel execution time.
TRACE_TMPDIR = None
LAST_EXEC_NS = None


def kernel(X, a_vals, Wn, bn, Wgi, bgi, Wgn, bgn, row, col):
    global LAST_EXEC_NS
    from concourse.bass_utils import run_bass_kernel_spmd

    per_core, (C0, C1) = _host_prep(X, a_vals, Wn, bn, Wgi, bgi, Wgn, bgn, row, col)
    if (C0, C1) not in _prog_cache:
        _prog_cache[(C0, C1)] = _build_program(C0, C1)
    nc = _prog_cache[(C0, C1)]

    kwargs = {}
    if TRACE_TMPDIR is not None:
        kwargs = {"trace": True, "tmpdir": TRACE_TMPDIR}
    res = run_bass_kernel_spmd(nc, per_core, core_ids=list(range(NC_)), **kwargs)
    LAST_EXEC_NS = res.exec_time_ns
    out = np.empty((N, D), np.float32)
    for k in range(NC_):
        out[k * NSH:(k + 1) * NSH] = res.results[k]["y"][:NSH]
    return out
